# revision 27
# baseline (speedup 1.0000x reference)
"""Multi-head attention (B=2, S=2048, D=1024, H=16, RoPE, full softmax) on
8 TRN2 NeuronCores.

Sharding: batch x head-group. Core c = 4*b + g handles batch b and heads
[4g, 4g+4). Each core computes q/k/v projections for its 4 heads, RoPE,
scores, softmax, attention, and a partial output projection against its
head-group's w_o columns. The host sums the 4 partial outputs per batch and
adds b_o.

Device layout highlights:
  - x is shipped transposed (xT [1024, 2048] bf16) so the d-contraction sits
    on partitions for both the q/k (w stationary) and v (x stationary)
    projections.
  - q/k weight rows are packed as TE/TO m-tiles (4 heads x 32 even dims,
    then odd dims) so RoPE becomes 4 fused (psum+bias)*table muls plus one
    add/sub per group, all partition-aligned.
  - scores are computed transposed (scoresT[t, s]) with head-PAIR row
    packing: kpair/qpair tiles hold two heads at partitions 0-63 / 64-127,
    so two K=64 matmuls run concurrently on disjoint array row-strips.
  - v carries an extra ones column per head: the attnT matmul's 65th output
    row accumulates the softmax denominator for free.
  - softmax skips max-subtraction (scores are pre-scaled by 1/8 via the RoPE
    tables; |scores| < ~7 so exp is safe in fp32->bf16).
  - every dma_start costs ~0.6us of serial dispatch on the issuing engine's
    sequencer queue, so DMAs are spread across sync/vector/scalar/gpsimd.
  - w_o and the q projections for s-chunks 1-3 are emitted as per-tile
    filler inside the exp-gated attention units to keep the PE busy (an
    idle PE drops from 2.4GHz to 1.2GHz p-state for ~3us).
"""

import os
import sys

for _p in ("/opt/trn_rl_repo",):
    if _p not in sys.path and os.path.isdir(_p):
        sys.path.append(_p)

import numpy as np
import ml_dtypes

import concourse.bass as bass
import concourse.mybir as mybir
from concourse.tile import TileContext
from concourse.bass_utils import run_bass_kernel_spmd

F32 = mybir.dt.float32
BF16 = mybir.dt.bfloat16
NPBF16 = ml_dtypes.bfloat16

B, S, D, H = 2, 2048, 1024, 16
HD = D // H          # 64
G = 4                # heads per core
P = 128
NCORES = 8
DC = D // P          # 8 d-chunks
ST = S // P          # 16 t-tiles
SC = S // 512        # 4 s-chunks of 512


# ---------------------------------------------------------------------------
# walrus workaround: this container's walrus rejects >1 sync wait per
# instruction. Hoist extra waits onto NoOps inserted just before the
# instruction on the same engine queue (queues execute in order, so this
# is semantics-preserving).
# ---------------------------------------------------------------------------
def _fix_multiwait(nc, max_waits=1):
    from bass_rust import SyncInfo

    n_split = 0
    for fn in nc.m.functions:
        for bb in fn.blocks:
            insts = bb.instructions
            out = []
            dirty = False
            for ins in insts:
                si = ins.sync_info
                if si is not None and si.on_wait and len(si.on_wait) > max_waits:
                    waits = list(si.on_wait)
                    for i, w in enumerate(waits[:-max_waits]):
                        nop = mybir.InstNoOp(name=f"{ins.name}-mw{i}")
                        nop.engine = ins.engine
                        nop.sync_info = SyncInfo(on_wait=[w], on_update=[])
                        out.append(nop)
                    ins.sync_info = SyncInfo(
                        on_wait=waits[-max_waits:], on_update=list(si.on_update)
                    )
                    dirty = True
                    n_split += 1
                out.append(ins)
            if dirty:
                bb.instructions = out
    return n_split


# ---------------------------------------------------------------------------
# device kernel
# ---------------------------------------------------------------------------
def _build_nc():
    # the exit drain's multi-wait is handled by _fix_multiwait (cheap NOPs)
    nc = bass.Bass()

    xT = nc.declare_dram_parameter("xT", [D, S], BF16, isOutput=False)
    wqkT = nc.declare_dram_parameter("wqkT", [D, 4 * P], BF16, isOutput=False)
    bqk = nc.declare_dram_parameter("bqk", [P, 4], F32, isOutput=False)
    wvT = nc.declare_dram_parameter("wvT", [D, G * HD], BF16, isOutput=False)
    bvb = nc.declare_dram_parameter("bvb", [P, G * HD], F32, isOutput=False)
    cosq = nc.declare_dram_parameter("cosq", [P, S], BF16, isOutput=False)
    sinq = nc.declare_dram_parameter("sinq", [P, S], BF16, isOutput=False)
    cosk = nc.declare_dram_parameter("cosk", [P, S], BF16, isOutput=False)
    sink = nc.declare_dram_parameter("sink", [P, S], BF16, isOutput=False)
    woT = nc.declare_dram_parameter("woT", [G * HD, D], BF16, isOutput=False)
    out = nc.declare_dram_parameter("out", [S, D], F32, isOutput=True)

    with TileContext(nc) as tc:
        with tc.tile_pool(name="const", bufs=1) as cpool:
            # ---- resident loads -------------------------------------------
            # Every DIRECT2D dma_start costs ~0.6us of serial dispatch on
            # the ISSUING engine's sequencer queue, so spread the loads
            # across idle queues (sync/vector/scalar/gpsimd) instead of
            # serializing ~30 dispatches on SP.
            xT_sb = cpool.tile([P, DC, S], BF16)
            wqk_sb = cpool.tile([P, DC, 4 * P], BF16)
            wv_sb = cpool.tile([P, DC, G * HD], BF16)
            xTr = xT[:].rearrange("(dc p) s -> p dc s", p=P)
            wqkr = wqkT[:].rearrange("(dc p) m -> p dc m", p=P)
            wvr = wvT[:].rearrange("(dc p) m -> p dc m", p=P)
            bqk_sb = cpool.tile([P, 4], F32)
            nc.scalar.dma_start(bqk_sb[:], bqk[:])
            for dc in range(DC):
                nc.sync.dma_start(wqk_sb[:, dc], wqkr[:, dc])
                nc.scalar.dma_start(xT_sb[:, dc, 0:512], xTr[:, dc, 0:512])
            for dc in range(DC):
                nc.scalar.dma_start(
                    xT_sb[:, dc, 512:1024], xTr[:, dc, 512:1024])
            tabs = {}
            for nm, src in (("cosk", cosk), ("sink", sink),
                            ("cosq", cosq), ("sinq", sinq)):
                t = cpool.tile([P, S], BF16, name=f"tab_{nm}")
                nc.gpsimd.dma_start(t[:], src[:])
                tabs[nm] = t
            nc.scalar.dma_start(wv_sb[:], wvr[:])
            bvb_sb = cpool.tile([P, G * HD], F32)
            nc.scalar.dma_start(bvb_sb[:], bvb[:])
            for dc in range(DC):
                nc.sync.dma_start(
                    xT_sb[:, dc, 1024:2048], xTr[:, dc, 1024:2048])
            wo_sb = cpool.tile([P, 2, D], BF16)
            nc.sync.dma_start(
                wo_sb[:], woT[:].rearrange("(jc p) d -> p jc d", p=P))

            # pair tiles (2 heads each at partitions 0-63 / 64-127)
            qpair = [cpool.tile([P, S], BF16, name=f"qpair{i}") for i in range(2)]
            kpair = [cpool.tile([P, S], BF16, name=f"kpair{i}") for i in range(2)]
            # v with ones column per head: the attnT matmul's 65th output
            # row accumulates the softmax denominator for free
            vext = cpool.tile([P, ST, G * 65], BF16)
            v4 = vext[:].rearrange("p t (h c) -> p t h c", c=65)
            nc.gpsimd.memset(v4[:, :, :, 64:65], 1.0)
            # normalized attention, assembled per pair [128 j, S] for w_o
            attn_n = [cpool.tile([P, S], BF16, name=f"attn{i}") for i in range(2)]
            # per-head raw/normalized attnT staging (base partition 0)
            attn_raw = [cpool.tile([HD + 1, S], BF16, name=f"attnraw{i}")
                        for i in range(4)]
            attn_nh = [cpool.tile([HD, S], BF16, name=f"attnnh{i}")
                       for i in range(4)]

            # ---- helpers --------------------------------------------------
            rtmp_cm = tc.tile_pool(name="rope_t", bufs=3)
            rtmp = rtmp_cm.__enter__()
            if True:
                def rope_group(grp, ps_pair, ssl, use_act=False,
                               dma_eng=None):
                    psTE, psTO = ps_pair
                    bTE = bqk_sb[:, 2 * grp:2 * grp + 1]
                    bTO = bqk_sb[:, 2 * grp + 1:2 * grp + 2]
                    cosT = tabs["cosq" if grp == 0 else "cosk"]
                    sinT = tabs["sinq" if grp == 0 else "sink"]
                    t1 = rtmp.tile([P, 512], BF16, tag="t1", name="t1")
                    t2 = rtmp.tile([P, 512], BF16, tag="t2", name="t2")
                    t3 = rtmp.tile([P, 512], BF16, tag="t3", name="t3")
                    t4 = rtmp.tile([P, 512], BF16, tag="t4", name="t4")
                    add, mult = mybir.AluOpType.add, mybir.AluOpType.mult
                    ident = mybir.ActivationFunctionType.Identity
                    if use_act:
                        # ACT idle window: evacuate the biased psum through
                        # Scalar, bf16 muls on DVE
                        eTE = rtmp.tile([P, 512], BF16, tag="eTE", name="eTE")
                        eTO = rtmp.tile([P, 512], BF16, tag="eTO", name="eTO")
                        nc.scalar.activation(eTE[:], psTE[:], ident, bias=bTE)
                        nc.scalar.activation(eTO[:], psTO[:], ident, bias=bTO)
                        nc.vector.tensor_mul(t1[:], eTE[:], cosT[:, ssl])
                        nc.vector.tensor_mul(t2[:], eTO[:], sinT[:, ssl])
                        nc.vector.tensor_mul(t3[:], eTE[:], sinT[:, ssl])
                        nc.vector.tensor_mul(t4[:], eTO[:], cosT[:, ssl])
                    else:
                        nc.vector.scalar_tensor_tensor(
                            t1[:], psTE[:], bTE, cosT[:, ssl], op0=add, op1=mult)
                        nc.vector.scalar_tensor_tensor(
                            t2[:], psTO[:], bTO, sinT[:, ssl], op0=add, op1=mult)
                        nc.vector.scalar_tensor_tensor(
                            t3[:], psTE[:], bTE, sinT[:, ssl], op0=add, op1=mult)
                        nc.vector.scalar_tensor_tensor(
                            t4[:], psTO[:], bTO, cosT[:, ssl], op0=add, op1=mult)
                    rotE = rtmp.tile([P, 512], BF16, tag="rotE", name="rotE")
                    rotO = rtmp.tile([P, 512], BF16, tag="rotO", name="rotO")
                    nc.vector.tensor_sub(rotE[:], t1[:], t2[:])
                    nc.vector.tensor_add(rotO[:], t3[:], t4[:])
                    dst = qpair if grp == 0 else kpair
                    eng = dma_eng if dma_eng is not None else nc.gpsimd
                    for pr in range(2):
                        for half, rot in ((0, rotE), (1, rotO)):
                            for hh in range(2):
                                src_lo = (2 * pr + hh) * 32
                                dst_lo = hh * 64 + half * 32
                                eng.dma_start(
                                    dst[pr][dst_lo:dst_lo + 32, ssl],
                                    rot[src_lo:src_lo + 32, :],
                                )

                def proj_mtile(m, ssl, pool, tag="o"):
                    ps = pool.tile([P, 512], F32, tag=tag, name="psqk")
                    for dc in range(DC):
                        nc.tensor.matmul(
                            ps[:],
                            wqk_sb[:, dc, m * P:(m + 1) * P],
                            xT_sb[:, dc, ssl],
                            start=(dc == 0), stop=(dc == DC - 1),
                        )
                    return ps

            # ---- projections + attention + w_o in one psum scope ----------
            with tc.tile_pool(name="ps_s", bufs=2, space="PSUM") as ps_sp, \
                 tc.tile_pool(name="ps_a", bufs=2, space="PSUM") as ps_ap, \
                 tc.tile_pool(name="ps_o", bufs=2, space="PSUM") as ps_op, \
                 tc.tile_pool(name="p_sb", bufs=8) as ppool, \
                 tc.tile_pool(name="norm", bufs=3) as npool, \
                 tc.tile_pool(name="dscr", bufs=4, space="DRAM") as dpool, \
                 tc.tile_pool(name="o_sb", bufs=3) as opool:
                def attn_groups(pr, sc, tts, psA, psB, filler=()):
                    fill_iter = iter(filler)
                    ssl = slice(sc * 512, (sc + 1) * 512)
                    for tt in tts:
                        pss = ps_sp.tile([P, 1024], F32, tag="sc", name="pss")
                        nc.tensor.matmul(
                            pss[:, 0:512],
                            kpair[pr][0:64, tt * P:(tt + 1) * P],
                            qpair[pr][0:64, ssl],
                            start=True, stop=True)
                        nc.tensor.matmul(
                            pss[:, 512:1024],
                            kpair[pr][64:128, tt * P:(tt + 1) * P],
                            qpair[pr][64:128, ssl],
                            start=True, stop=True)
                        p_sb = ppool.tile([P, 1024], BF16, tag="p", name="p_sb")
                        nc.scalar.activation(
                            p_sb[:], pss[:], mybir.ActivationFunctionType.Exp)
                        hA, hB = 2 * pr, 2 * pr + 1
                        nc.tensor.matmul(
                            psA[:],
                            vext[:, tt, hA * 65:hA * 65 + 65],
                            p_sb[:, 0:512],
                            start=(tt == 0), stop=(tt == ST - 1))
                        nc.tensor.matmul(
                            psB[:],
                            vext[:, tt, hB * 65:hB * 65 + 65],
                            p_sb[:, 512:1024],
                            start=(tt == 0), stop=(tt == ST - 1))
                        step = next(fill_iter, None)
                        if step is not None:
                            step()
                    for step in fill_iter:
                        step()

                def attn_norm_tail(pr, sc, psA, psB):
                    # no DMA bounce: direct [1,512] reciprocal of both
                    # halves first (the serial latency sits at the kernel
                    # end), then shuffle-broadcast + normalize
                    ssl = slice(sc * 512, (sc + 1) * 512)
                    den2 = npool.tile([HD, 512], F32, tag="den2", name="den2")
                    nc.vector.tensor_copy(den2[0:1, :], psA[64:65, :])
                    nc.vector.tensor_copy(den2[32:33, :], psB[64:65, :])
                    rc1 = npool.tile([HD, 512], F32, tag="rc1", name="rc1")
                    nc.vector.reciprocal(rc1[0:33, :], den2[0:33, :])
                    for hh, psX in ((0, psA), (1, psB)):
                        h = 2 * pr + hh
                        bc = npool.tile([HD, 512], F32, tag="bc", name="bc")
                        nc.vector.stream_shuffle(
                            bc[0:32, :], rc1[32 * hh:32 * hh + 32, :],
                            mask=[0] * 32)
                        nc.vector.stream_shuffle(
                            bc[32:64, :], rc1[32 * hh:32 * hh + 32, :],
                            mask=[0] * 32)
                        nc.vector.tensor_mul(
                            attn_nh[h][:, ssl], psX[0:64, :], bc[:])
                        nc.sync.dma_start(
                            attn_n[pr][hh * 64:(hh + 1) * 64, ssl],
                            attn_nh[h][:, ssl])

                def attn_norm(pr, sc, psA, psB):
                    # normalize straight out of PSUM: spread the 65th row
                    # across partitions by DMA (DVE reciprocal cost scales
                    # with free-dim size, not partitions), reciprocal,
                    # gather back, broadcast with stream_shuffle
                    ssl = slice(sc * 512, (sc + 1) * 512)
                    for hh, psX in ((0, psA), (1, psB)):
                        h = 2 * pr + hh
                        nc.vector.tensor_copy(attn_raw[h][:, ssl], psX[:, :])
                        dr1 = dpool.tile([512], BF16, tag="dr1", name="dr1")
                        nc.sync.dma_start(dr1[:], attn_raw[h][64:65, ssl])
                        dsc = npool.tile([P, 4], BF16, tag="dsc", name="dsc")
                        nc.sync.dma_start(
                            dsc[:], dr1[:].rearrange("(p c) -> p c", p=P))
                        drc = npool.tile([P, 4], F32, tag="drc", name="drc")
                        nc.vector.reciprocal(drc[:], dsc[:])
                        dr2 = dpool.tile([512], F32, tag="dr2", name="dr2")
                        nc.sync.dma_start(
                            dr2[:].rearrange("(p c) -> p c", p=P), drc[:])
                        dg = npool.tile([32, 512], F32, tag="dg", name="dg")
                        nc.sync.dma_start(dg[0:1, :], dr2[:])
                        bc = npool.tile([HD, 512], F32, tag="bc", name="bc")
                        nc.vector.stream_shuffle(
                            bc[0:32, :], dg[:, :], mask=[0] * 32)
                        nc.vector.stream_shuffle(
                            bc[32:64, :], dg[:, :], mask=[0] * 32)
                        nc.vector.tensor_mul(
                            attn_nh[h][:, ssl], attn_raw[h][0:64, ssl], bc[:])
                        nc.sync.dma_start(
                            attn_n[pr][hh * 64:(hh + 1) * 64, ssl],
                            attn_nh[h][:, ssl])

                # ---- filler step generators --------------------------------
                # One step ~= one PE matmul (~210ns at full clock). Each
                # attention tile leaves a ~200-300ns PE bubble while Scalar's
                # exp gates the next attn matmul; feeding exactly one filler
                # matmul per tile keeps the PE continuously busy, which also
                # holds it at the 2.4GHz p-state (any idle drops it to
                # 1.2GHz for ~3us — the dominant baseline loss).
                def qproj_filler(sc):
                    # 8 steps x 2 matmuls: q m-tiles 0,1 for s-chunk sc,
                    # then RoPE. Concentrated in the FIRST tiles of the
                    # preceding unit so qpair is ready well before the
                    # consuming unit starts (spreading it across all 16
                    # tiles starved the next unit's score stream).
                    ssl = slice(sc * 512, (sc + 1) * 512)
                    state = {}

                    def mk(m, dc):
                        def emit():
                            if dc == 0:
                                state[m] = ps_op.tile(
                                    [P, 512], F32, tag="o", name=f"psq{m}")
                            nc.tensor.matmul(
                                state[m][:],
                                wqk_sb[:, dc, m * P:(m + 1) * P],
                                xT_sb[:, dc, ssl],
                                start=(dc == 0), stop=(dc == DC - 1),
                            )
                            if m == 1 and dc == DC - 1:
                                with tc.high_priority():
                                    rope_group(0, [state[0], state[1]], ssl,
                                               dma_eng=nc.sync)
                        return emit

                    steps = [mk(m, dc) for m in (0, 1) for dc in range(DC)]

                    def pair(a, b):
                        def emit():
                            a()
                            b()
                        return emit

                    return [pair(steps[2 * i], steps[2 * i + 1])
                            for i in range(len(steps) // 2)]

                def wo_filler(sc):
                    # 16 steps: w_o for this s-chunk's 4 s-tiles
                    state = {}

                    def mk(st, half, jc):
                        def emit():
                            key = (st, half)
                            if jc == 0:
                                state[key] = ps_op.tile(
                                    [P, 512], F32, tag="o", name="pso")
                            nc.tensor.matmul(
                                state[key][:],
                                attn_n[jc][:, st * P:(st + 1) * P],
                                wo_sb[:, jc, half * 512:(half + 1) * 512],
                                start=(jc == 0), stop=(jc == 1))
                            if jc == 1:
                                osb = opool.tile(
                                    [P, 512], F32, tag="ot", name="osb")
                                nc.vector.tensor_copy(osb[:], state[key][:])
                                nc.sync.dma_start(
                                    out[st * P:(st + 1) * P,
                                        half * 512:(half + 1) * 512],
                                    osb[:])
                        return emit

                    return [mk(st, half, jc)
                            for st in range(sc * 4, sc * 4 + 4)
                            for half in range(2) for jc in range(2)]

                # projections: k, q, v per s-chunk through the tag-"o"
                # psum ring. Attention for (pair 0, s-chunk 0) is emitted
                # inside the production loop so the exp stream starts early.
                # preload the exp ACT table while input DMAs land so the
                # first real exp doesn't pay the ~2.7us table-load
                warm = rtmp.tile([1, 16], F32, tag="t1", name="warm")
                nc.vector.memset(warm[:], 0.0)
                warm2 = rtmp.tile([1, 16], BF16, tag="t2", name="warm2")
                nc.scalar.activation(warm2[:], warm[:],
                                     mybir.ActivationFunctionType.Exp)
                psA0 = ps_ap.tile([65, 512], F32, tag="at", name="psA0")
                psB0 = ps_ap.tile([65, 512], F32, tag="at", name="psB0")
                for sc in range(SC):
                    ssl = slice(sc * 512, (sc + 1) * 512)
                    rope_group(
                        1,
                        [proj_mtile(2, ssl, ps_op),
                         proj_mtile(3, ssl, ps_op)],
                        ssl, use_act=(sc == 0))
                    if sc == 0:
                        rope_group(
                            0,
                            [proj_mtile(0, ssl, ps_op),
                             proj_mtile(1, ssl, ps_op)],
                            ssl)
                    for tl in range(4):
                        tt = sc * 4 + tl
                        psv = ps_op.tile([P, G * HD], F32, tag="o", name="psv")
                        for dc in range(DC):
                            nc.tensor.matmul(
                                psv[:],
                                xT_sb[:, dc, tt * P:(tt + 1) * P],
                                wv_sb[:, dc, :],
                                start=(dc == 0), stop=(dc == DC - 1),
                            )
                        nc.vector.tensor_add(
                            vext[:, tt, :].rearrange(
                                "p (h c) -> p h c", c=65)[:, :, 0:64],
                            psv[:, :].rearrange("p (h c) -> p h c", c=HD),
                            bvb_sb[:, :].rearrange("p (h c) -> p h c", c=HD),
                        )
                    if sc >= 1:
                        attn_groups(
                            0, 0, range((sc - 1) * 4, sc * 4), psA0, psB0)
                attn_groups(0, 0, range(12, 16), psA0, psB0)
                attn_norm(0, 0, psA0, psB0)

                # attention units in s-chunk-major order: each s-chunk's
                # w_o becomes per-tile filler two units later (keeps the PE
                # p-state up); q-projections for later s-chunks run in the
                # first 8 tiles of the unit one earlier.
                wo0, wo1, wo2 = wo_filler(0), wo_filler(1), wo_filler(2)
                units = [
                    (1, 0, qproj_filler(1)),
                    (0, 1, qproj_filler(2) + wo0[0:8]),
                    (1, 1, wo0[8:16]),
                    (0, 2, qproj_filler(3) + wo1[0:8]),
                    (1, 2, wo1[8:16]),
                    (0, 3, wo2[0:8]),
                    (1, 3, wo2[8:16]),
                ]
                for i, (pr, sc, fill) in enumerate(units):
                    psA = ps_ap.tile([65, 512], F32, tag="at", name="psA")
                    psB = ps_ap.tile([65, 512], F32, tag="at", name="psB")
                    attn_groups(pr, sc, range(ST), psA, psB, filler=fill)
                    if i == len(units) - 1:
                        attn_norm_tail(pr, sc, psA, psB)
                    else:
                        attn_norm(pr, sc, psA, psB)
                for step in wo_filler(3):
                    step()

            rtmp_cm.__exit__(None, None, None)

    _fix_multiwait(nc)
    return nc


_NC_CACHE = None


def _get_nc():
    global _NC_CACHE
    if _NC_CACHE is None:
        _NC_CACHE = _build_nc()
    return _NC_CACHE


# ---------------------------------------------------------------------------
# host-side sharding
# ---------------------------------------------------------------------------
def _deint(rows):
    """rows [64, ...] -> [even dims (32); odd dims (32)]"""
    return np.concatenate([rows[0::2], rows[1::2]], axis=0)


def _shard_inputs(input, rotations, w_qkv, b_qkv, w_o, b_o):
    x = np.asarray(input, np.float32)
    rot = np.asarray(rotations, np.float32)
    w_qkv = np.asarray(w_qkv, np.float32)
    b_qkv = np.asarray(b_qkv, np.float32)
    w_o = np.asarray(w_o, np.float32)

    cos = rot[:, :, 0].T.copy()   # [32, S]
    sin = rot[:, :, 1].T.copy()
    cos4 = np.tile(cos, (4, 1))   # [128, S]
    sin4 = np.tile(sin, (4, 1))
    cosq_h = (cos4 / 8.0).astype(NPBF16)
    sinq_h = (sin4 / 8.0).astype(NPBF16)
    cosk_h = cos4.astype(NPBF16)
    sink_h = sin4.astype(NPBF16)

    in_maps = []
    for c in range(NCORES):
        b, g = divmod(c, 4)
        heads = [4 * g + i for i in range(G)]
        xT = np.ascontiguousarray(x[b].T).astype(NPBF16)          # [D, S]

        # q/k m-tiles: TE then TO, 4 heads x 32 rows each, for q then k
        q_te, q_to, k_te, k_to, bq_te, bq_to, bk_te, bk_to = \
            [], [], [], [], [], [], [], []
        for h in heads:
            qw = _deint(w_qkv[h * HD:(h + 1) * HD])
            kw = _deint(w_qkv[D + h * HD:D + (h + 1) * HD])
            qb = _deint(b_qkv[h * HD:(h + 1) * HD])
            kb = _deint(b_qkv[D + h * HD:D + (h + 1) * HD])
            q_te.append(qw[:32]); q_to.append(qw[32:])
            k_te.append(kw[:32]); k_to.append(kw[32:])
            bq_te.append(qb[:32]); bq_to.append(qb[32:])
            bk_te.append(kb[:32]); bk_to.append(kb[32:])
        wqk = np.concatenate(
            [np.concatenate(blk, axis=0) for blk in (q_te, q_to, k_te, k_to)],
            axis=0)                                                # [512, D]
        wqkT = np.ascontiguousarray(wqk.T).astype(NPBF16)          # [D, 512]
        bqk = np.stack(
            [np.concatenate(blk) for blk in (bq_te, bq_to, bk_te, bk_to)],
            axis=1).astype(np.float32)                             # [128, 4]

        wv = np.concatenate(
            [w_qkv[2 * D + h * HD:2 * D + (h + 1) * HD] for h in heads], axis=0)
        wvT = np.ascontiguousarray(wv.T).astype(NPBF16)            # [D, 256]
        bv = np.concatenate(
            [b_qkv[2 * D + h * HD:2 * D + (h + 1) * HD] for h in heads])
        bvb = np.tile(bv[None, :], (P, 1)).astype(np.float32)      # [128, 256]

        wo = w_o[:, g * G * HD:(g + 1) * G * HD]                   # [D, 256]
        woT = np.ascontiguousarray(wo.T).astype(NPBF16)            # [256, D]

        in_maps.append({
            "xT": xT, "wqkT": wqkT, "bqk": bqk, "wvT": wvT, "bvb": bvb,
            "cosq": cosq_h, "sinq": sinq_h, "cosk": cosk_h, "sink": sink_h,
            "woT": woT,
        })
    return in_maps


def _run(inputs, trace=False):
    nc = _get_nc()
    in_maps = _shard_inputs(**inputs)
    res = run_bass_kernel_spmd(
        nc, in_maps, core_ids=list(range(NCORES)), trace=trace)
    b_o = np.asarray(inputs["b_o"], np.float32)
    out = np.zeros((B, S, D), np.float32)
    for c in range(NCORES):
        out[c // 4] += res.results[c]["out"]
    out += b_o[None, None, :]
    return out, res


def kernel(**inputs):
    out, _ = _run(inputs, trace=False)
    return out



# revision 28
# speedup vs baseline: 1.0254x; 1.0254x over previous
"""Multi-head attention (B=2, S=2048, D=1024, H=16, RoPE, full softmax) on
8 TRN2 NeuronCores.

Sharding: batch x head-group. Core c = 4*b + g handles batch b and heads
[4g, 4g+4). Each core computes q/k/v projections for its 4 heads, RoPE,
scores, softmax, attention, and a partial output projection against its
head-group's w_o columns. The host sums the 4 partial outputs per batch and
adds b_o.

Device layout highlights:
  - x is shipped transposed (xT [1024, 2048] bf16) so the d-contraction sits
    on partitions for both the q/k (w stationary) and v (x stationary)
    projections.
  - q/k weight rows are packed as TE/TO m-tiles (4 heads x 32 even dims,
    then odd dims) so RoPE becomes 4 fused (psum+bias)*table muls plus one
    add/sub per group, all partition-aligned.
  - scores are computed transposed (scoresT[t, s]) with head-PAIR row
    packing: kpair/qpair tiles hold two heads at partitions 0-63 / 64-127,
    so two K=64 matmuls run concurrently on disjoint array row-strips.
  - v carries an extra ones column per head: the attnT matmul's 65th output
    row accumulates the softmax denominator for free.
  - softmax skips max-subtraction (scores are pre-scaled by 1/8 via the RoPE
    tables; |scores| < ~7 so exp is safe in fp32->bf16).
  - every dma_start costs ~0.6us of serial dispatch on the issuing engine's
    sequencer queue, so DMAs are spread across sync/vector/scalar/gpsimd.
  - w_o and the q projections for s-chunks 1-3 are emitted as per-tile
    filler inside the exp-gated attention units to keep the PE busy (an
    idle PE drops from 2.4GHz to 1.2GHz p-state for ~3us).
"""

import os
import sys

for _p in ("/opt/trn_rl_repo",):
    if _p not in sys.path and os.path.isdir(_p):
        sys.path.append(_p)

import numpy as np
import ml_dtypes

import concourse.bass as bass
import concourse.mybir as mybir
from concourse.tile import TileContext
from concourse.bass_utils import run_bass_kernel_spmd

F32 = mybir.dt.float32
BF16 = mybir.dt.bfloat16
NPBF16 = ml_dtypes.bfloat16

B, S, D, H = 2, 2048, 1024, 16
HD = D // H          # 64
G = 4                # heads per core
P = 128
NCORES = 8
DC = D // P          # 8 d-chunks
ST = S // P          # 16 t-tiles
SC = S // 512        # 4 s-chunks of 512


# ---------------------------------------------------------------------------
# walrus workaround: this container's walrus rejects >1 sync wait per
# instruction. Hoist extra waits onto NoOps inserted just before the
# instruction on the same engine queue (queues execute in order, so this
# is semantics-preserving).
# ---------------------------------------------------------------------------
def _fix_multiwait(nc, max_waits=1):
    from bass_rust import SyncInfo

    n_split = 0
    for fn in nc.m.functions:
        for bb in fn.blocks:
            insts = bb.instructions
            out = []
            dirty = False
            for ins in insts:
                si = ins.sync_info
                if si is not None and si.on_wait and len(si.on_wait) > max_waits:
                    waits = list(si.on_wait)
                    for i, w in enumerate(waits[:-max_waits]):
                        nop = mybir.InstNoOp(name=f"{ins.name}-mw{i}")
                        nop.engine = ins.engine
                        nop.sync_info = SyncInfo(on_wait=[w], on_update=[])
                        out.append(nop)
                    ins.sync_info = SyncInfo(
                        on_wait=waits[-max_waits:], on_update=list(si.on_update)
                    )
                    dirty = True
                    n_split += 1
                out.append(ins)
            if dirty:
                bb.instructions = out
    return n_split


# ---------------------------------------------------------------------------
# device kernel
# ---------------------------------------------------------------------------
def _build_nc():
    # the exit drain's multi-wait is handled by _fix_multiwait (cheap NOPs)
    nc = bass.Bass()

    xT = nc.declare_dram_parameter("xT", [D, S], BF16, isOutput=False)
    wqkT = nc.declare_dram_parameter("wqkT", [D, 4 * P], BF16, isOutput=False)
    bqk = nc.declare_dram_parameter("bqk", [P, 4], F32, isOutput=False)
    wvT = nc.declare_dram_parameter("wvT", [D, G * HD], BF16, isOutput=False)
    bvb = nc.declare_dram_parameter("bvb", [P, G * HD], F32, isOutput=False)
    cosq = nc.declare_dram_parameter("cosq", [P, S], BF16, isOutput=False)
    sinq = nc.declare_dram_parameter("sinq", [P, S], BF16, isOutput=False)
    cosk = nc.declare_dram_parameter("cosk", [P, S], BF16, isOutput=False)
    sink = nc.declare_dram_parameter("sink", [P, S], BF16, isOutput=False)
    woT = nc.declare_dram_parameter("woT", [G * HD, D], BF16, isOutput=False)
    out = nc.declare_dram_parameter("out", [S, D], F32, isOutput=True)

    with TileContext(nc) as tc:
        with tc.tile_pool(name="const", bufs=1) as cpool:
            # ---- resident loads -------------------------------------------
            # Every DIRECT2D dma_start costs ~0.6us of serial dispatch on
            # the ISSUING engine's sequencer queue, so spread the loads
            # across idle queues (sync/vector/scalar/gpsimd) instead of
            # serializing ~30 dispatches on SP.
            xT_sb = cpool.tile([P, DC, S], BF16)
            wqk_sb = cpool.tile([P, DC, 4 * P], BF16)
            wv_sb = cpool.tile([P, DC, G * HD], BF16)
            xTr = xT[:].rearrange("(dc p) s -> p dc s", p=P)
            wqkr = wqkT[:].rearrange("(dc p) m -> p dc m", p=P)
            wvr = wvT[:].rearrange("(dc p) m -> p dc m", p=P)
            bqk_sb = cpool.tile([P, 4], F32)
            nc.scalar.dma_start(bqk_sb[:], bqk[:])
            for dc in range(DC):
                nc.sync.dma_start(wqk_sb[:, dc], wqkr[:, dc])
                nc.scalar.dma_start(xT_sb[:, dc, 0:512], xTr[:, dc, 0:512])
            for dc in range(DC):
                nc.scalar.dma_start(
                    xT_sb[:, dc, 512:1024], xTr[:, dc, 512:1024])
            tabs = {}
            for nm, src in (("cosk", cosk), ("sink", sink),
                            ("cosq", cosq), ("sinq", sinq)):
                t = cpool.tile([P, S], BF16, name=f"tab_{nm}")
                nc.gpsimd.dma_start(t[:], src[:])
                tabs[nm] = t
            nc.scalar.dma_start(wv_sb[:], wvr[:])
            bvb_sb = cpool.tile([P, G * HD], F32)
            nc.scalar.dma_start(bvb_sb[:], bvb[:])
            for dc in range(DC):
                nc.sync.dma_start(
                    xT_sb[:, dc, 1024:2048], xTr[:, dc, 1024:2048])
            wo_sb = cpool.tile([P, 2, D], BF16)
            nc.sync.dma_start(
                wo_sb[:], woT[:].rearrange("(jc p) d -> p jc d", p=P))

            # pair tiles (2 heads each at partitions 0-63 / 64-127)
            qpair = [cpool.tile([P, S], BF16, name=f"qpair{i}") for i in range(2)]
            kpair = [cpool.tile([P, S], BF16, name=f"kpair{i}") for i in range(2)]
            # v with ones column per head: the attnT matmul's 65th output
            # row accumulates the softmax denominator for free
            vext = cpool.tile([P, ST, G * 65], BF16)
            v4 = vext[:].rearrange("p t (h c) -> p t h c", c=65)
            nc.gpsimd.memset(v4[:, :, :, 64:65], 1.0)
            # normalized attention, assembled per pair [128 j, S] for w_o
            attn_n = [cpool.tile([P, S], BF16, name=f"attn{i}") for i in range(2)]
            # per-head raw/normalized attnT staging (base partition 0)
            attn_raw = [cpool.tile([HD + 1, S], BF16, name=f"attnraw{i}")
                        for i in range(4)]
            attn_nh = [cpool.tile([HD, S], BF16, name=f"attnnh{i}")
                       for i in range(4)]

            # ---- helpers --------------------------------------------------
            rtmp_cm = tc.tile_pool(name="rope_t", bufs=3)
            rtmp = rtmp_cm.__enter__()
            if True:
                def rope_group(grp, ps_pair, ssl, use_act=False,
                               dma_eng=None):
                    psTE, psTO = ps_pair
                    bTE = bqk_sb[:, 2 * grp:2 * grp + 1]
                    bTO = bqk_sb[:, 2 * grp + 1:2 * grp + 2]
                    cosT = tabs["cosq" if grp == 0 else "cosk"]
                    sinT = tabs["sinq" if grp == 0 else "sink"]
                    t1 = rtmp.tile([P, 512], BF16, tag="t1", name="t1")
                    t2 = rtmp.tile([P, 512], BF16, tag="t2", name="t2")
                    t3 = rtmp.tile([P, 512], BF16, tag="t3", name="t3")
                    t4 = rtmp.tile([P, 512], BF16, tag="t4", name="t4")
                    add, mult = mybir.AluOpType.add, mybir.AluOpType.mult
                    ident = mybir.ActivationFunctionType.Identity
                    if use_act:
                        # ACT idle window: evacuate the biased psum through
                        # Scalar, bf16 muls on DVE
                        eTE = rtmp.tile([P, 512], BF16, tag="eTE", name="eTE")
                        eTO = rtmp.tile([P, 512], BF16, tag="eTO", name="eTO")
                        nc.scalar.activation(eTE[:], psTE[:], ident, bias=bTE)
                        nc.scalar.activation(eTO[:], psTO[:], ident, bias=bTO)
                        nc.vector.tensor_mul(t1[:], eTE[:], cosT[:, ssl])
                        nc.vector.tensor_mul(t2[:], eTO[:], sinT[:, ssl])
                        nc.vector.tensor_mul(t3[:], eTE[:], sinT[:, ssl])
                        nc.vector.tensor_mul(t4[:], eTO[:], cosT[:, ssl])
                    else:
                        nc.vector.scalar_tensor_tensor(
                            t1[:], psTE[:], bTE, cosT[:, ssl], op0=add, op1=mult)
                        nc.vector.scalar_tensor_tensor(
                            t2[:], psTO[:], bTO, sinT[:, ssl], op0=add, op1=mult)
                        nc.vector.scalar_tensor_tensor(
                            t3[:], psTE[:], bTE, sinT[:, ssl], op0=add, op1=mult)
                        nc.vector.scalar_tensor_tensor(
                            t4[:], psTO[:], bTO, cosT[:, ssl], op0=add, op1=mult)
                    rotE = rtmp.tile([P, 512], BF16, tag="rotE", name="rotE")
                    rotO = rtmp.tile([P, 512], BF16, tag="rotO", name="rotO")
                    nc.vector.tensor_sub(rotE[:], t1[:], t2[:])
                    nc.vector.tensor_add(rotO[:], t3[:], t4[:])
                    dst = qpair if grp == 0 else kpair
                    eng = dma_eng if dma_eng is not None else nc.gpsimd
                    for pr in range(2):
                        for half, rot in ((0, rotE), (1, rotO)):
                            for hh in range(2):
                                src_lo = (2 * pr + hh) * 32
                                dst_lo = hh * 64 + half * 32
                                eng.dma_start(
                                    dst[pr][dst_lo:dst_lo + 32, ssl],
                                    rot[src_lo:src_lo + 32, :],
                                )

                def proj_mtile(m, ssl, pool, tag="o"):
                    ps = pool.tile([P, 512], F32, tag=tag, name="psqk")
                    for dc in range(DC):
                        nc.tensor.matmul(
                            ps[:],
                            wqk_sb[:, dc, m * P:(m + 1) * P],
                            xT_sb[:, dc, ssl],
                            start=(dc == 0), stop=(dc == DC - 1),
                        )
                    return ps

            # ---- projections + attention + w_o in one psum scope ----------
            with tc.tile_pool(name="ps_s", bufs=2, space="PSUM") as ps_sp, \
                 tc.tile_pool(name="ps_a", bufs=2, space="PSUM") as ps_ap, \
                 tc.tile_pool(name="ps_o", bufs=2, space="PSUM") as ps_op, \
                 tc.tile_pool(name="p_sb", bufs=8) as ppool, \
                 tc.tile_pool(name="norm", bufs=3) as npool, \
                 tc.tile_pool(name="dscr", bufs=4, space="DRAM") as dpool, \
                 tc.tile_pool(name="o_sb", bufs=3) as opool:
                def attn_groups(pr, sc, tts, psA, psB, filler=()):
                    fill_iter = iter(filler)
                    ssl = slice(sc * 512, (sc + 1) * 512)
                    for tt in tts:
                        pss = ps_sp.tile([P, 1024], F32, tag="sc", name="pss")
                        nc.tensor.matmul(
                            pss[:, 0:512],
                            kpair[pr][0:64, tt * P:(tt + 1) * P],
                            qpair[pr][0:64, ssl],
                            start=True, stop=True)
                        nc.tensor.matmul(
                            pss[:, 512:1024],
                            kpair[pr][64:128, tt * P:(tt + 1) * P],
                            qpair[pr][64:128, ssl],
                            start=True, stop=True)
                        p_sb = ppool.tile([P, 1024], BF16, tag="p", name="p_sb")
                        nc.scalar.activation(
                            p_sb[:], pss[:], mybir.ActivationFunctionType.Exp)
                        hA, hB = 2 * pr, 2 * pr + 1
                        nc.tensor.matmul(
                            psA[:],
                            vext[:, tt, hA * 65:hA * 65 + 65],
                            p_sb[:, 0:512],
                            start=(tt == 0), stop=(tt == ST - 1))
                        nc.tensor.matmul(
                            psB[:],
                            vext[:, tt, hB * 65:hB * 65 + 65],
                            p_sb[:, 512:1024],
                            start=(tt == 0), stop=(tt == ST - 1))
                        step = next(fill_iter, None)
                        if step is not None:
                            step()
                    for step in fill_iter:
                        step()

                def attn_norm_tail(pr, sc, psA, psB):
                    # no DMA bounce: direct [1,512] reciprocal of both
                    # halves first (the serial latency sits at the kernel
                    # end), then shuffle-broadcast + normalize
                    ssl = slice(sc * 512, (sc + 1) * 512)
                    den2 = npool.tile([HD, 512], F32, tag="den2", name="den2")
                    nc.vector.tensor_copy(den2[0:1, :], psA[64:65, :])
                    nc.vector.tensor_copy(den2[32:33, :], psB[64:65, :])
                    rc1 = npool.tile([HD, 512], F32, tag="rc1", name="rc1")
                    nc.vector.reciprocal(rc1[0:33, :], den2[0:33, :])
                    for hh, psX in ((0, psA), (1, psB)):
                        h = 2 * pr + hh
                        bc = npool.tile([HD, 512], F32, tag="bc", name="bc")
                        nc.vector.stream_shuffle(
                            bc[0:32, :], rc1[32 * hh:32 * hh + 32, :],
                            mask=[0] * 32)
                        nc.vector.stream_shuffle(
                            bc[32:64, :], rc1[32 * hh:32 * hh + 32, :],
                            mask=[0] * 32)
                        nc.vector.tensor_mul(
                            attn_nh[h][:, ssl], psX[0:64, :], bc[:])
                        nc.sync.dma_start(
                            attn_n[pr][hh * 64:(hh + 1) * 64, ssl],
                            attn_nh[h][:, ssl])

                def attn_norm(pr, sc, psA, psB):
                    # normalize straight out of PSUM: spread the 65th row
                    # across partitions by DMA (DVE reciprocal cost scales
                    # with free-dim size, not partitions), reciprocal,
                    # gather back, broadcast with stream_shuffle
                    ssl = slice(sc * 512, (sc + 1) * 512)
                    for hh, psX in ((0, psA), (1, psB)):
                        h = 2 * pr + hh
                        nc.vector.tensor_copy(attn_raw[h][:, ssl], psX[:, :])
                        dr1 = dpool.tile([512], BF16, tag="dr1", name="dr1")
                        nc.sync.dma_start(dr1[:], attn_raw[h][64:65, ssl])
                        dsc = npool.tile([P, 4], BF16, tag="dsc", name="dsc")
                        nc.sync.dma_start(
                            dsc[:], dr1[:].rearrange("(p c) -> p c", p=P))
                        drc = npool.tile([P, 4], F32, tag="drc", name="drc")
                        nc.vector.reciprocal(drc[:], dsc[:])
                        dr2 = dpool.tile([512], F32, tag="dr2", name="dr2")
                        nc.sync.dma_start(
                            dr2[:].rearrange("(p c) -> p c", p=P), drc[:])
                        dg = npool.tile([32, 512], F32, tag="dg", name="dg")
                        nc.sync.dma_start(dg[0:1, :], dr2[:])
                        bc = npool.tile([HD, 512], F32, tag="bc", name="bc")
                        nc.vector.stream_shuffle(
                            bc[0:32, :], dg[:, :], mask=[0] * 32)
                        nc.vector.stream_shuffle(
                            bc[32:64, :], dg[:, :], mask=[0] * 32)
                        nc.vector.tensor_mul(
                            attn_nh[h][:, ssl], attn_raw[h][0:64, ssl], bc[:])
                        nc.sync.dma_start(
                            attn_n[pr][hh * 64:(hh + 1) * 64, ssl],
                            attn_nh[h][:, ssl])

                # ---- filler step generators --------------------------------
                # One step ~= one PE matmul (~210ns at full clock). Each
                # attention tile leaves a ~200-300ns PE bubble while Scalar's
                # exp gates the next attn matmul; feeding exactly one filler
                # matmul per tile keeps the PE continuously busy, which also
                # holds it at the 2.4GHz p-state (any idle drops it to
                # 1.2GHz for ~3us — the dominant baseline loss).
                def qproj_filler(sc):
                    # 8 steps x 2 matmuls: q m-tiles 0,1 for s-chunk sc,
                    # then RoPE. Concentrated in the FIRST tiles of the
                    # preceding unit so qpair is ready well before the
                    # consuming unit starts (spreading it across all 16
                    # tiles starved the next unit's score stream).
                    ssl = slice(sc * 512, (sc + 1) * 512)
                    state = {}

                    def mk(m, dc):
                        def emit():
                            if dc == 0:
                                state[m] = ps_op.tile(
                                    [P, 512], F32, tag="o", name=f"psq{m}")
                            nc.tensor.matmul(
                                state[m][:],
                                wqk_sb[:, dc, m * P:(m + 1) * P],
                                xT_sb[:, dc, ssl],
                                start=(dc == 0), stop=(dc == DC - 1),
                            )
                            if m == 1 and dc == DC - 1:
                                rope_group(0, [state[0], state[1]], ssl,
                                           dma_eng=nc.sync)
                        return emit

                    steps = [mk(m, dc) for m in (0, 1) for dc in range(DC)]

                    def pair(a, b):
                        def emit():
                            a()
                            b()
                        return emit

                    return [pair(steps[2 * i], steps[2 * i + 1])
                            for i in range(len(steps) // 2)]

                def wo_filler(sc):
                    # 16 steps: w_o for this s-chunk's 4 s-tiles
                    state = {}

                    def mk(st, half, jc):
                        def emit():
                            key = (st, half)
                            if jc == 0:
                                state[key] = ps_op.tile(
                                    [P, 512], F32, tag="o", name="pso")
                            nc.tensor.matmul(
                                state[key][:],
                                attn_n[jc][:, st * P:(st + 1) * P],
                                wo_sb[:, jc, half * 512:(half + 1) * 512],
                                start=(jc == 0), stop=(jc == 1))
                            if jc == 1:
                                osb = opool.tile(
                                    [P, 512], F32, tag="ot", name="osb")
                                nc.vector.tensor_copy(osb[:], state[key][:])
                                nc.sync.dma_start(
                                    out[st * P:(st + 1) * P,
                                        half * 512:(half + 1) * 512],
                                    osb[:])
                        return emit

                    return [mk(st, half, jc)
                            for st in range(sc * 4, sc * 4 + 4)
                            for half in range(2) for jc in range(2)]

                # projections: k, q, v per s-chunk through the tag-"o"
                # psum ring. Attention for (pair 0, s-chunk 0) is emitted
                # inside the production loop so the exp stream starts early.
                # preload the exp ACT table while input DMAs land so the
                # first real exp doesn't pay the ~2.7us table-load
                warm = rtmp.tile([1, 16], F32, tag="t1", name="warm")
                nc.vector.memset(warm[:], 0.0)
                warm2 = rtmp.tile([1, 16], BF16, tag="t2", name="warm2")
                nc.scalar.activation(warm2[:], warm[:],
                                     mybir.ActivationFunctionType.Exp)
                psA0 = ps_ap.tile([65, 512], F32, tag="at", name="psA0")
                psB0 = ps_ap.tile([65, 512], F32, tag="at", name="psB0")
                for sc in range(SC):
                    ssl = slice(sc * 512, (sc + 1) * 512)
                    rope_group(
                        1,
                        [proj_mtile(2, ssl, ps_op),
                         proj_mtile(3, ssl, ps_op)],
                        ssl, use_act=(sc == 0))
                    if sc == 0:
                        rope_group(
                            0,
                            [proj_mtile(0, ssl, ps_op),
                             proj_mtile(1, ssl, ps_op)],
                            ssl)
                    for tl in range(4):
                        tt = sc * 4 + tl
                        psv = ps_op.tile([P, G * HD], F32, tag="o", name="psv")
                        for dc in range(DC):
                            nc.tensor.matmul(
                                psv[:],
                                xT_sb[:, dc, tt * P:(tt + 1) * P],
                                wv_sb[:, dc, :],
                                start=(dc == 0), stop=(dc == DC - 1),
                            )
                        nc.vector.tensor_add(
                            vext[:, tt, :].rearrange(
                                "p (h c) -> p h c", c=65)[:, :, 0:64],
                            psv[:, :].rearrange("p (h c) -> p h c", c=HD),
                            bvb_sb[:, :].rearrange("p (h c) -> p h c", c=HD),
                        )
                    if sc >= 1:
                        attn_groups(
                            0, 0, range((sc - 1) * 4, sc * 4), psA0, psB0)
                attn_groups(0, 0, range(12, 16), psA0, psB0)
                attn_norm(0, 0, psA0, psB0)

                # attention units in s-chunk-major order: each s-chunk's
                # w_o becomes per-tile filler two units later (keeps the PE
                # p-state up); q-projections for later s-chunks run in the
                # first 8 tiles of the unit one earlier.
                wo0, wo1, wo2 = wo_filler(0), wo_filler(1), wo_filler(2)
                units = [
                    (1, 0, qproj_filler(1)),
                    (0, 1, wo0[0:8]),
                    (1, 1, qproj_filler(2) + wo0[8:16]),
                    (0, 2, wo1[0:8]),
                    (1, 2, qproj_filler(3) + wo1[8:16]),
                    (0, 3, wo2[0:8]),
                    (1, 3, wo2[8:16]),
                ]
                for i, (pr, sc, fill) in enumerate(units):
                    psA = ps_ap.tile([65, 512], F32, tag="at", name="psA")
                    psB = ps_ap.tile([65, 512], F32, tag="at", name="psB")
                    attn_groups(pr, sc, range(ST), psA, psB, filler=fill)
                    if i == len(units) - 1:
                        attn_norm_tail(pr, sc, psA, psB)
                    else:
                        attn_norm(pr, sc, psA, psB)
                for step in wo_filler(3):
                    step()

            rtmp_cm.__exit__(None, None, None)

    _fix_multiwait(nc)
    return nc


_NC_CACHE = None


def _get_nc():
    global _NC_CACHE
    if _NC_CACHE is None:
        _NC_CACHE = _build_nc()
    return _NC_CACHE


# ---------------------------------------------------------------------------
# host-side sharding
# ---------------------------------------------------------------------------
def _deint(rows):
    """rows [64, ...] -> [even dims (32); odd dims (32)]"""
    return np.concatenate([rows[0::2], rows[1::2]], axis=0)


def _shard_inputs(input, rotations, w_qkv, b_qkv, w_o, b_o):
    x = np.asarray(input, np.float32)
    rot = np.asarray(rotations, np.float32)
    w_qkv = np.asarray(w_qkv, np.float32)
    b_qkv = np.asarray(b_qkv, np.float32)
    w_o = np.asarray(w_o, np.float32)

    cos = rot[:, :, 0].T.copy()   # [32, S]
    sin = rot[:, :, 1].T.copy()
    cos4 = np.tile(cos, (4, 1))   # [128, S]
    sin4 = np.tile(sin, (4, 1))
    cosq_h = (cos4 / 8.0).astype(NPBF16)
    sinq_h = (sin4 / 8.0).astype(NPBF16)
    cosk_h = cos4.astype(NPBF16)
    sink_h = sin4.astype(NPBF16)

    in_maps = []
    for c in range(NCORES):
        b, g = divmod(c, 4)
        heads = [4 * g + i for i in range(G)]
        xT = np.ascontiguousarray(x[b].T).astype(NPBF16)          # [D, S]

        # q/k m-tiles: TE then TO, 4 heads x 32 rows each, for q then k
        q_te, q_to, k_te, k_to, bq_te, bq_to, bk_te, bk_to = \
            [], [], [], [], [], [], [], []
        for h in heads:
            qw = _deint(w_qkv[h * HD:(h + 1) * HD])
            kw = _deint(w_qkv[D + h * HD:D + (h + 1) * HD])
            qb = _deint(b_qkv[h * HD:(h + 1) * HD])
            kb = _deint(b_qkv[D + h * HD:D + (h + 1) * HD])
            q_te.append(qw[:32]); q_to.append(qw[32:])
            k_te.append(kw[:32]); k_to.append(kw[32:])
            bq_te.append(qb[:32]); bq_to.append(qb[32:])
            bk_te.append(kb[:32]); bk_to.append(kb[32:])
        wqk = np.concatenate(
            [np.concatenate(blk, axis=0) for blk in (q_te, q_to, k_te, k_to)],
            axis=0)                                                # [512, D]
        wqkT = np.ascontiguousarray(wqk.T).astype(NPBF16)          # [D, 512]
        bqk = np.stack(
            [np.concatenate(blk) for blk in (bq_te, bq_to, bk_te, bk_to)],
            axis=1).astype(np.float32)                             # [128, 4]

        wv = np.concatenate(
            [w_qkv[2 * D + h * HD:2 * D + (h + 1) * HD] for h in heads], axis=0)
        wvT = np.ascontiguousarray(wv.T).astype(NPBF16)            # [D, 256]
        bv = np.concatenate(
            [b_qkv[2 * D + h * HD:2 * D + (h + 1) * HD] for h in heads])
        bvb = np.tile(bv[None, :], (P, 1)).astype(np.float32)      # [128, 256]

        wo = w_o[:, g * G * HD:(g + 1) * G * HD]                   # [D, 256]
        woT = np.ascontiguousarray(wo.T).astype(NPBF16)            # [256, D]

        in_maps.append({
            "xT": xT, "wqkT": wqkT, "bqk": bqk, "wvT": wvT, "bvb": bvb,
            "cosq": cosq_h, "sinq": sinq_h, "cosk": cosk_h, "sink": sink_h,
            "woT": woT,
        })
    return in_maps


def _run(inputs, trace=False):
    nc = _get_nc()
    in_maps = _shard_inputs(**inputs)
    res = run_bass_kernel_spmd(
        nc, in_maps, core_ids=list(range(NCORES)), trace=trace)
    b_o = np.asarray(inputs["b_o"], np.float32)
    out = np.zeros((B, S, D), np.float32)
    for c in range(NCORES):
        out[c // 4] += res.results[c]["out"]
    out += b_o[None, None, :]
    return out, res


def kernel(**inputs):
    out, _ = _run(inputs, trace=False)
    return out

